# revision 1
# baseline (speedup 1.0000x reference)
"""Trainium2 Bass kernel for nn_LossFunction_46720654246163.

Contrastive (SimCLR-style) loss over N=8192 rows:
  feat = concat(view0, view1) rows, fn = feat / ||feat||
  S = fn @ fn.T  [N,N];  logits = w*S + b;  masked softmax per row
  loss = mean_i [ -(logits[i,pos]-m_i) + log(sum_{j!=i} exp(logits[i,j]-m_i)) ]
  prec1 = 100 * mean_i [ argmax_{j!=i} logits[i,j] == pos(i) ],  pos(i)=(i+N/2)%N

Row-parallel across 8 NeuronCores; the host rotates row order per core so all
cores run the IDENTICAL program (own rows at columns [0,1024), positives at a
fixed +4096 offset). Scalar means are order-invariant -> no un-rotation.

Per core:
 - sumsq via DVE bn_stats, rnorm = exp(-0.5*ln(ss)) on ACT,
 - normalize+transpose fused: regular matmul  featT_norm = nat.T @ diag(rn)
   (diag built by GPSIMD affine_select), rounded to float32r on copy-out,
 - per 128-row M-tile: float32r matmuls into PSUM (q order 2,0,1,3),
   S_pos extracted from the q=2 block, per-row bias = -w*S_pos,
 - ONE ACT pass per psum tile: E' = exp(w*S - w*S_pos_i) with fused row-sum.
   Softmax-shift invariance makes loss_i = ln(sum E') exactly; the self column
   is pre-masked by an accumulating (-BIG*I) matmul so it contributes 0.
 - prec1: row-max of E' (fp16 TT-max tree) vs 1.01: the positive term is
   exp(~0)=1, any competitor above it exceeds e^{w*margin} >~ 1.03.
 - ACT activation tables are pinned to the single set that holds
   {exp, ln, square, copy} so there is exactly one ACT_TABLE_LOAD.
"""
import numpy as np
from contextlib import ExitStack

import concourse.bass as bass
import concourse.tile as tile
from concourse import bacc, mybir
from concourse import hw_specs
from concourse.bass_utils import run_bass_kernel_spmd

F32 = mybir.dt.float32
F32R = mybir.dt.float32r
F16 = mybir.dt.float16
AF = mybir.ActivationFunctionType
ALU = mybir.AluOpType

N_CORES = 8
B, C, D = 4096, 2, 128
N = B * C
ROWS = N // N_CORES
MT = ROWS // 128               # 8 M-tiles per core
JT = N // 512                  # 16 column tiles of 512
QT = 4                         # psum rounds per M-tile ([128,2048] each)
POS_OFF = N // 2
NEG_BIG = 1.0e5
CORR_THR = 1.01

# --- tuning knobs ---
NCHUNK = 8                     # phase-1 chunks (8 nat tiles of 128 rows each)
COPY_ACT = 16                  # fnT psum->sbuf copies on ACT (of 16)
TREE_STOP = 512                # TT-max tree -> tensor_reduce switch width
SQ_ACT_CHUNKS = 0              # early chunks' sumsq on ACT Square (rest: DVE bn_stats)
DIAG_TRANSPOSE = True          # fuse normalize into transpose via diag(rn) matmul

_cache = {}
_act_tables_patched = False


def _pin_act_tables():
    """Force every activation in this process onto the one table set that
    contains exp+ln+square+copy, so bacc emits a single ACT_TABLE_LOAD."""
    global _act_tables_patched
    if _act_tables_patched:
        return
    orig = hw_specs.get_activation_tables
    keep = "natural_log_exp_and_others"
    pin = {AF.Exp, AF.Ln, AF.Square, AF.Copy, AF.Identity}

    def patched(arch):
        tabs = orig(arch)
        if keep not in tabs:
            return tabs
        return {name: (funcs if name == keep else funcs - pin)
                for name, funcs in tabs.items()}

    hw_specs.get_activation_tables = patched
    bacc.get_activation_tables = patched
    _act_tables_patched = True


def _build_program(w: float, b: float):
    _pin_act_tables()
    nc = bacc.Bacc("TRN2", target_bir_lowering=False, debug=False,
                   enable_asserts=True, num_devices=N_CORES)

    d_feat = nc.dram_tensor("feat", [N, D], F32, kind="ExternalInput").ap()
    d_identf = nc.dram_tensor("identf", [128, 128], F32, kind="ExternalInput").ap()
    d_negbig = nc.dram_tensor("negbig", [128, 128], F32, kind="ExternalInput").ap()
    o_loss = nc.dram_tensor("loss_out", [128, MT], F32, kind="ExternalOutput").ap()
    o_corr = nc.dram_tensor("corr_out", [128, MT], F32, kind="ExternalOutput").ap()

    TPC = 64 // NCHUNK

    with tile.TileContext(nc) as tc, ExitStack() as ctx:
        consts = ctx.enter_context(tc.tile_pool(name="consts", bufs=1))
        natp = ctx.enter_context(tc.tile_pool(name="nat", bufs=1))
        fntp = ctx.enter_context(tc.tile_pool(name="fnt", bufs=1))
        stats = ctx.enter_context(tc.tile_pool(name="stats", bufs=1))
        scrp = ctx.enter_context(tc.tile_pool(name="scr", bufs=2))
        diagp = ctx.enter_context(tc.tile_pool(name="diag", bufs=8))
        ep = ctx.enter_context(tc.tile_pool(name="ep", bufs=2))
        treep = ctx.enter_context(tc.tile_pool(name="tree", bufs=3))
        psum = ctx.enter_context(tc.tile_pool(name="psum", bufs=2, space="PSUM"))

        identf = consts.tile([128, 128], F32, tag="identf")
        negbig = consts.tile([128, 128], F32, tag="negbig")
        nc.sync.dma_start(out=identf[:], in_=d_identf)
        nc.sync.dma_start(out=negbig[:], in_=d_negbig)
        identr = consts.tile([128, 128], F32R, tag="identr")
        nc.vector.tensor_copy(identr[:], identf[:])
        negbigr = consts.tile([128, 128], F32R, tag="negbigr")
        nc.vector.tensor_copy(negbigr[:], negbig[:])

        ss = stats.tile([128, 64], F32, tag="ss")
        lnss = stats.tile([128, 64], F32, tag="lnss")
        rn = stats.tile([128, 64], F32, tag="rn")
        mvall = stats.tile([128, 64, 2], F32, tag="mvall")
        zacc = stats.tile([128, MT * QT], F32, tag="zacc")
        spos = stats.tile([128, MT], F32, tag="spos")
        biasm = stats.tile([128, MT], F32, tag="biasm")
        emax = stats.tile([128, MT], F32, tag="emax")

        feat3 = d_feat.rearrange("(c t p) d -> c p t d", c=NCHUNK, t=TPC)

        # ---------- phase 1: load, sumsq, rnorm ----------
        # chunk order: the m-loop consumes q=2 (fnt[8..11] <- chunks 4,5) first,
        # then q=0 (chunks 0,1), q=1 (2,3), q=3 (6,7) -- produce in that order.
        CHUNK_ORDER = [0, 4, 5, 1, 2, 3, 6, 7]
        J_ORDER = [0, 1, 8, 9, 10, 11, 2, 3, 4, 5, 6, 7, 12, 13, 14, 15]
        nat = [None] * NCHUNK
        for cch in CHUNK_ORDER:
            nchunk = natp.tile([128, TPC, 128], F32, tag=f"nat{cch}")
            nc.sync.dma_start(out=nchunk[:], in_=feat3[cch])
            nat[cch] = nchunk
            sl = slice(cch * TPC, (cch + 1) * TPC)
            if CHUNK_ORDER.index(cch) < SQ_ACT_CHUNKS:
                # early chunks: ACT is idle, Square+accum there
                for t in range(TPC):
                    g = cch * TPC + t
                    sq_scr = scrp.tile([128, 128], F32, tag="sq_scr")
                    nc.scalar.activation(out=sq_scr[:], in_=nchunk[:, t, :],
                                         func=AF.Square, accum_out=ss[:, g:g + 1])
                nc.vector.tensor_scalar_max(ss[:, sl], ss[:, sl], 1e-16)
            else:
                for t in range(TPC):
                    g = cch * TPC + t
                    bns = scrp.tile([128, 6], F32, tag="bns")
                    nc.vector.bn_stats(out=bns[:], in_=nchunk[:, t, :])
                    nc.vector.bn_aggr(out=mvall[:, g, :], in_=bns[:])
                # ss = D * (mean^2 + var)
                m2 = scrp.tile([128, TPC], F32, tag="m2")
                nc.vector.tensor_tensor(out=m2[:], in0=mvall[:, sl, 0],
                                        in1=mvall[:, sl, 0], op=ALU.mult)
                nc.vector.tensor_tensor(out=m2[:], in0=m2[:],
                                        in1=mvall[:, sl, 1], op=ALU.add)
                nc.vector.tensor_scalar(out=ss[:, sl], in0=m2[:], scalar1=float(D),
                                        scalar2=1e-16, op0=ALU.mult, op1=ALU.max)
            # rn = ss^-1/2 = exp(-0.5*ln(ss))
            nc.scalar.activation(out=lnss[:, sl], in_=ss[:, sl], func=AF.Ln)
            nc.scalar.activation(out=rn[:, sl], in_=lnss[:, sl], func=AF.Exp,
                                 bias=0.0, scale=-0.5)

        # ---------- transpose (+normalize) to fnT float32r ----------
        # Emitted lazily (right before first use) so PSUM pool slots interleave
        # between transpose tiles and main matmul tiles instead of serializing.
        fnt = {}
        state = {"n_copy_act": 0}

        def ensure_fnt(j):
            if j in fnt:
                return fnt[j]
            cch, t0 = divmod(4 * j, TPC)
            pt = psum.tile([128, 2048], F32, tag="psum")
            for q in range(4):
                t = t0 + q
                g = cch * TPC + t
                if DIAG_TRANSPOSE:
                    dt_ = diagp.tile([128, 128], F32, tag="dt")
                    nc.gpsimd.affine_select(
                        out=dt_[:], in_=rn[:, g:g + 1].to_broadcast((128, 128)),
                        compare_op=ALU.is_equal, fill=0.0, base=0,
                        pattern=[[-1, 128]], channel_multiplier=1)
                    nc.tensor.matmul(pt[:, q * 128:(q + 1) * 128],
                                     nat[cch][:, t, :], dt_[:],
                                     start=True, stop=True)
                else:
                    nc.vector.tensor_scalar_mul(nat[cch][:, t, :],
                                                nat[cch][:, t, :], rn[:, g:g + 1])
                    nc.tensor.transpose(pt[:, q * 128:(q + 1) * 128],
                                        nat[cch][:, t, :], identf[:])
            ftile = fntp.tile([128, 512], F32R, tag=f"fnt{j}")
            # early copies gate the m-loop; ACT is idle early, so they go there
            if state["n_copy_act"] < COPY_ACT:
                nc.scalar.copy(ftile[:], pt[:, 0:512])
                state["n_copy_act"] += 1
            else:
                nc.vector.tensor_copy(ftile[:], pt[:, 0:512])
            fnt[j] = ftile
            return ftile

        # ---------- phase 2: S block, bias shift, exp+sum, max ----------
        for m in range(MT):
            lhsT = ensure_fnt(m // 4)[:, (m % 4) * 128:(m % 4 + 1) * 128]
            etile = ep.tile([128, N], F16, tag="E")
            for q in (2, 0, 1, 3):
                for jj in range(4):
                    ensure_fnt(4 * q + jj)
                pm = psum.tile([128, 2048], F32, tag="psum")
                # in q=2 the pos-block matmul goes first so the S_pos extract
                # (and the exp bias) unblocks as early as possible
                jjs = [m // 4] + [x for x in range(4) if x != m // 4] if q == 2 else range(4)
                for jj in jjs:
                    j = 4 * q + jj
                    nc.tensor.matmul(pm[:, jj * 512:(jj + 1) * 512], lhsT, fnt[j][:],
                                     start=True, stop=True)
                if q == 2:
                    # positive at col 4096+128m -> offset 128m within q=2
                    pscr = scrp.tile([128, 128], F32, tag="pscr")
                    nc.vector.tensor_tensor(out=pscr[:],
                                            in0=pm[:, 128 * m:128 * (m + 1)],
                                            in1=identf[:], op=ALU.mult)
                    nc.vector.tensor_reduce(out=spos[:, m:m + 1], in_=pscr[:],
                                            axis=mybir.AxisListType.X, op=ALU.add)
                    nc.vector.tensor_scalar_mul(biasm[:, m:m + 1], spos[:, m:m + 1],
                                                -w)
                if q == 0:
                    # self column block: accumulate -BIG*I
                    nc.tensor.matmul(pm[:, 128 * m:128 * (m + 1)], identr[:],
                                     negbigr[:], start=False, stop=True,
                                     skip_group_check=True)
                nc.scalar.activation(out=etile[:, q * 2048:(q + 1) * 2048], in_=pm[:],
                                     func=AF.Exp, bias=biasm[:, m:m + 1], scale=w,
                                     accum_out=zacc[:, QT * m + q:QT * m + q + 1])
                # running row-max right after each 2048-block is produced
                if q == 0:
                    rmax = treep.tile([128, 2048], F16, tag="rmax")
                    nc.vector.tensor_tensor(out=rmax[:], in0=etile[:, 4096:6144],
                                            in1=etile[:, 0:2048], op=ALU.max)
                elif q != 2:
                    nc.vector.tensor_tensor(out=rmax[:], in0=rmax[:],
                                            in1=etile[:, q * 2048:(q + 1) * 2048],
                                            op=ALU.max)
            prev = rmax
            width = 1024
            while width >= TREE_STOP:
                tt = treep.tile([128, width], F16, tag=f"tree{width}")
                nc.vector.tensor_tensor(out=tt[:], in0=prev[:, 0:width],
                                        in1=prev[:, width:2 * width], op=ALU.max)
                prev = tt
                width //= 2
            nc.vector.tensor_reduce(out=emax[:, m:m + 1], in_=prev[:],
                                    axis=mybir.AxisListType.X, op=ALU.max)

        # ---------- finals ----------
        z = stats.tile([128, MT], F32, tag="z")
        nc.vector.tensor_reduce(out=z[:], in_=zacc[:].rearrange("p (m q) -> p m q", q=QT),
                                axis=mybir.AxisListType.X, op=ALU.add)
        lossb = stats.tile([128, MT], F32, tag="lossb")
        nc.scalar.activation(out=lossb[:], in_=z[:], func=AF.Ln)
        corrb = stats.tile([128, MT], F32, tag="corrb")
        nc.vector.tensor_scalar(out=corrb[:], in0=emax[:], scalar1=CORR_THR,
                                scalar2=None, op0=ALU.is_le)
        nc.sync.dma_start(out=o_loss, in_=lossb[:])
        nc.sync.dma_start(out=o_corr, in_=corrb[:])

    nc.compile()
    return nc


def _get_program(w: float, b: float):
    key = (w, b)
    if key not in _cache:
        _cache[key] = _build_program(w, b)
    return _cache[key]


def make_in_maps(features: np.ndarray):
    feat = np.ascontiguousarray(np.swapaxes(np.asarray(features, np.float32), 0, 1).reshape(N, D))
    identf = np.eye(128, dtype=np.float32)
    negbig = (-NEG_BIG * np.eye(128)).astype(np.float32)
    in_maps = []
    for c in range(N_CORES):
        rot = np.roll(feat, -ROWS * c, axis=0) if c else feat
        in_maps.append({"feat": np.ascontiguousarray(rot), "identf": identf,
                        "negbig": negbig})
    return in_maps


def kernel(features: np.ndarray, w: np.ndarray, b: np.ndarray):
    features = np.asarray(features, dtype=np.float32)
    wf = float(np.asarray(w)); bf = float(np.asarray(b))
    assert features.shape == (B, C, D), features.shape

    nc = _get_program(wf, bf)
    in_maps = make_in_maps(features)
    res = run_bass_kernel_spmd(nc, in_maps, list(range(N_CORES)))

    loss_sum = 0.0
    corr_sum = 0.0
    for c in range(N_CORES):
        loss_sum += float(res.results[c]["loss_out"].astype(np.float64).sum())
        corr_sum += float(res.results[c]["corr_out"].astype(np.float64).sum())
    return (np.float32(loss_sum / N), np.float32(100.0 * corr_sum / N))


if __name__ == "__main__":
    import jax
    key = jax.random.key(0)
    k1, = jax.random.split(key, 1)
    feats = np.asarray(jax.random.normal(k1, (B, C, D), dtype=np.float32))
    out = kernel(features=feats, w=np.float32(10.0), b=np.float32(-5.0))
    print("loss, prec1 =", out)

